# revision 4
# baseline (speedup 1.0000x reference)
"""HGT-style GNN message passing on 8 Trainium2 NeuronCores.

Strategy: dst-node sharding. Host packs the 50000 dst nodes into 392 blocks of
128 slots (degree-balanced), 49 blocks per core; every edge lives with its dst
block so edge-softmax segment ops are block-local (no scatter anywhere).
Projections k/v/q are computed replicated on every core into one interleaved
bf16 table [NPAD, 384]; per-edge rows are fetched with indirect DMA gathers
(k+v share one 512B descriptor). Segment max is skipped (t = dot/4 stays in a
safe exp range; softmax is shift-invariant) and segment sums are PE matmuls
against per-chunk one-hot matrices. h_out shards are AllGathered in bf16 for
the final edge-score gathers.
"""

import numpy as np
import ml_dtypes

BF16 = ml_dtypes.bfloat16

H = 8
DK = 16
OUT = 128
NCORES = 8
P = 128


def _set_scale(n, e, nblk_c):
    global N, E, NBLK_C, NBLK, NPAD, SE_REAL, SCOLS, SE
    N, E, NBLK_C = n, e, nblk_c
    NBLK = NCORES * NBLK_C
    NPAD = NBLK * P
    SE_REAL = E // NCORES
    SCOLS = -(-SE_REAL // P)          # idx columns per polarity
    SE = SCOLS * P


_set_scale(50000, 800000, 49)


# ---------------------------------------------------------------- host prep

def _pack_graph(dst):
    """Assign nodes to (core, block, slot) balancing block edge counts."""
    deg = np.bincount(dst, minlength=N)
    order = np.argsort(-deg, kind="stable")
    i = np.arange(N)
    r, pos = i // NBLK, i % NBLK
    blk_of_rank = np.where(r % 2 == 0, pos, NBLK - 1 - pos)
    blk_of_node = np.empty(N, np.int64)
    blk_of_node[order] = blk_of_rank
    load = np.zeros(NBLK, np.int64)
    np.add.at(load, blk_of_node[dst], 1)
    border = np.argsort(-load, kind="stable")
    j = np.arange(NBLK)
    rr, pp = j // NCORES, j % NCORES
    core_of_rank = np.where(rr % 2 == 0, pp, NCORES - 1 - pp)
    core_of_blk = np.empty(NBLK, np.int64)
    core_of_blk[border] = core_of_rank
    locblk_of_blk = np.empty(NBLK, np.int64)
    for c in range(NCORES):
        mine = np.where(core_of_blk == c)[0]
        locblk_of_blk[mine] = np.arange(len(mine))
    slot_of_node = np.empty(N, np.int64)
    node_of_row = np.full(NPAD, -1, np.int64)
    for b in range(NBLK):
        nodes = np.where(blk_of_node == b)[0]
        slot_of_node[nodes] = np.arange(len(nodes))
        row0 = core_of_blk[b] * NBLK_C * P + locblk_of_blk[b] * P
        node_of_row[row0 + np.arange(len(nodes))] = nodes
    prow = (core_of_blk[blk_of_node] * NBLK_C * P
            + locblk_of_blk[blk_of_node] * P + slot_of_node)
    return blk_of_node, core_of_blk, locblk_of_blk, slot_of_node, prow, \
        node_of_row, load


def _prep(h, Wq, bq, Wk, bk, Wv, bv, Wmsg, bmsg, Wattn, battn, Wa, ba,
          src, dst, neg_src, neg_dst):
    h = np.asarray(h, np.float32)
    src = np.asarray(src)
    dst = np.asarray(dst)
    (blk_of_node, core_of_blk, locblk_of_blk, slot_of_node, prow,
     node_of_row, load) = _pack_graph(dst)
    T = max(2, int(np.ceil(load.max() / P)))  # chunks per block

    Wkk = (np.asarray(Wk) @ np.asarray(Wattn)).astype(np.float32)
    bkk = (np.asarray(bk) @ np.asarray(Wattn) + battn).astype(np.float32)
    Wvv = (np.asarray(Wv) @ np.asarray(Wmsg)).astype(np.float32)
    bvv = (np.asarray(bv) @ np.asarray(Wmsg) + bmsg).astype(np.float32)
    w_kvq = np.concatenate([Wkk, Wvv, np.asarray(Wq, np.float32)], axis=1)
    b_kvq = np.concatenate([bkk, bvv, np.asarray(bq, np.float32)])[None, :]

    hT = np.zeros((128, NPAD), np.float32)
    valid = node_of_row >= 0
    hT[:, valid] = h[node_of_row[valid]].T

    eblk = blk_of_node[dst]
    src_w = np.zeros((NCORES, 128, NBLK_C * T), np.int32)
    qdst_w = np.zeros((NCORES, 128, NBLK_C * T), np.int32)
    dloc_w = np.full((NCORES, 128, NBLK_C * T), 255.0, np.float32)
    order_e = np.lexsort((np.arange(E), eblk))
    eb_sorted = eblk[order_e]
    starts = np.searchsorted(eb_sorted, np.arange(NBLK))
    ends = np.searchsorted(eb_sorted, np.arange(NBLK) + 1)
    for b in range(NBLK):
        es = order_e[starts[b]:ends[b]]
        c, lb = core_of_blk[b], locblk_of_blk[b]
        j = np.arange(len(es))
        cols = lb * T + j // P
        rows = j % P
        src_w[c, rows, cols] = prow[src[es]]
        qdst_w[c, rows, cols] = prow[dst[es]]
        dloc_w[c, rows, cols] = slot_of_node[dst[es]]

    def score_idx(arr):
        arr = np.asarray(arr)
        out = np.zeros((NCORES, 128, SCOLS), np.int32)
        for c in range(NCORES):
            buf = np.zeros(SE, np.int64)
            buf[:SE_REAL] = prow[arr[c * SE_REAL:(c + 1) * SE_REAL]]
            out[c] = buf.reshape(SCOLS, 128).T
        return out

    sp_s, sp_d = score_idx(src), score_idx(dst)
    sn_s, sn_d = score_idx(neg_src), score_idx(neg_dst)

    consts = dict(
        w_kvq=w_kvq.astype(BF16), b_kvq=b_kvq.astype(BF16),
        wa=np.asarray(Wa, np.float32).astype(BF16),
        ba=np.asarray(ba, np.float32)[None, :].astype(BF16),
        iota=np.broadcast_to(np.arange(128, dtype=np.float32),
                             (128, 128)).copy(),
        identb=np.eye(128, dtype=np.float32).astype(BF16),
        identf=np.eye(128, dtype=np.float32),
        ones1=np.ones((1, 128), np.float32).astype(BF16),
        hT=hT.astype(BF16),
    )
    per_core = []
    for c in range(NCORES):
        per_core.append(dict(
            src_idx=src_w[c], qdst_idx=qdst_w[c],
            dstloc=dloc_w[c],
            sp_src=sp_s[c], sp_dst=sp_d[c],
            sn_src=sn_s[c], sn_dst=sn_d[c],
        ))
    meta = dict(T=T, node_of_row=node_of_row)
    return consts, per_core, meta


# ---------------------------------------------------------------- program

_PROG_CACHE = {}


def _bcast_last(ap, count):
    """Append a broadcast (step 0) innermost dim to an AP view."""
    import concourse.bass as bass
    return bass.AP(ap.tensor, ap.offset,
                   [list(x) for x in ap.ap] + [[0, count]])


def _build_program(T):
    import concourse.bacc as bacc
    import concourse.bass as bass
    import concourse.tile as tile
    import concourse.mybir as mybir

    dt = mybir.dt
    AT = mybir.AluOpType
    AF = mybir.ActivationFunctionType

    nc = bacc.Bacc("TRN2", target_bir_lowering=False, debug=False,
                   num_devices=NCORES)

    def din(name, shape, d):
        return nc.dram_tensor(name, shape, d, kind="ExternalInput")

    def dout(name, shape, d):
        return nc.dram_tensor(name, shape, d, kind="ExternalOutput")

    hT = din("hT", [128, NPAD], dt.bfloat16)
    w_kvq = din("w_kvq", [128, 384], dt.bfloat16)
    b_kvq = din("b_kvq", [1, 384], dt.bfloat16)
    wa = din("wa", [128, 128], dt.bfloat16)
    ba = din("ba", [1, 128], dt.bfloat16)
    iota = din("iota", [128, 128], dt.float32)
    identb = din("identb", [128, 128], dt.bfloat16)
    identf = din("identf", [128, 128], dt.float32)
    ones1 = din("ones1", [1, 128], dt.bfloat16)
    src_idx = din("src_idx", [128, NBLK_C * T], dt.int32)
    qdst_idx = din("qdst_idx", [128, NBLK_C * T], dt.int32)
    dstloc = din("dstloc", [128, NBLK_C * T], dt.float32)
    sp_src = din("sp_src", [128, SCOLS], dt.int32)
    sp_dst = din("sp_dst", [128, SCOLS], dt.int32)
    sn_src = din("sn_src", [128, SCOLS], dt.int32)
    sn_dst = din("sn_dst", [128, SCOLS], dt.int32)

    hout_f32 = dout("hout_f32", [NBLK_C * P, 128], dt.float32)
    scores_pos = dout("scores_pos", [SCOLS, 128], dt.float32)
    scores_neg = dout("scores_neg", [SCOLS, 128], dt.float32)

    kvq_table = nc.dram_tensor("kvq_table", [NPAD, 384], dt.bfloat16)
    hout_sh = nc.dram_tensor("hout_sh", [NBLK_C * P, 128], dt.bfloat16)
    hout_full = nc.dram_tensor("hout_full", [NPAD, 128], dt.bfloat16,
                               addr_space="Shared")

    IOA = bass.IndirectOffsetOnAxis

    with tile.TileContext(nc) as tc:
        with tc.tile_pool(name="const", bufs=1) as cp:
            def cload(name, shape, d, srcap):
                t = cp.tile(shape, d, tag=name)
                nc.sync.dma_start(out=t[:], in_=srcap)
                return t

            wkvq_sb = cload("wkvq", [128, 384], dt.bfloat16, w_kvq[:, :])
            bkvq_sb = cload("bkvq", [1, 384], dt.bfloat16, b_kvq[:, :])
            wa_sb = cload("wa", [128, 128], dt.bfloat16, wa[:, :])
            ba_sb = cload("ba", [1, 128], dt.bfloat16, ba[:, :])
            iota_sb = cload("iota", [128, 128], dt.float32, iota[:, :])
            identb_sb = cload("identb", [128, 128], dt.bfloat16, identb[:, :])
            identf_sb = cload("identf", [128, 128], dt.float32, identf[:, :])
            ones_sb = cload("ones1", [1, 128], dt.bfloat16, ones1[:, :])
            srcidx_sb = cload("srcidx", [128, NBLK_C * T], dt.int32,
                              src_idx[:, :])
            qdst_sb = cload("qdsti", [128, NBLK_C * T], dt.int32,
                            qdst_idx[:, :])
            dloc_sb = cload("dloc", [128, NBLK_C * T], dt.float32,
                            dstloc[:, :])
            spsrc_sb = cload("spsrc", [128, SCOLS], dt.int32, sp_src[:, :])
            spdst_sb = cload("spdst", [128, SCOLS], dt.int32, sp_dst[:, :])
            snsrc_sb = cload("snsrc", [128, SCOLS], dt.int32, sn_src[:, :])
            sndst_sb = cload("sndst", [128, SCOLS], dt.int32, sn_dst[:, :])

            # ---------------- phase A: kvq projection table (replicated)
            with tc.tile_pool(name="pa", bufs=3) as pa, \
                    tc.tile_pool(name="paps", bufs=4, space="PSUM") as paps:
                for j in range(NPAD // 512):
                    ht_sb = pa.tile([128, 512], dt.bfloat16, tag="ht")
                    nc.sync.dma_start(out=ht_sb[:],
                                      in_=hT[:, j * 512:(j + 1) * 512])
                    for k in range(4):
                        r = j * 4 + k
                        ps = paps.tile([128, 384], dt.float32, tag="pskvq")
                        nc.tensor.matmul(out=ps[:],
                                         lhsT=ht_sb[:, k * 128:(k + 1) * 128],
                                         rhs=wkvq_sb[:], start=True,
                                         stop=False)
                        nc.tensor.matmul(out=ps[:], lhsT=ones_sb[:],
                                         rhs=bkvq_sb[:], start=False,
                                         stop=True)
                        kv_sb = pa.tile([128, 384], dt.bfloat16, tag="kvo")
                        if r % 2 == 0:
                            nc.vector.tensor_copy(out=kv_sb[:], in_=ps[:])
                        else:
                            nc.scalar.activation(kv_sb[:], ps[:], AF.Copy)
                        nc.sync.dma_start(
                            out=kvq_table[r * 128:(r + 1) * 128, :],
                            in_=kv_sb[:])

            tc.strict_bb_all_engine_barrier()

            # ---------------- phase B: per dst-block edge processing
            with tc.tile_pool(name="pb", bufs=2) as pb, \
                    tc.tile_pool(name="pbm", bufs=3) as pbm, \
                    tc.tile_pool(name="pbps", bufs=2, space="PSUM") as pbps:
                for b in range(NBLK_C):
                    ic0 = b * T
                    kv_t = pb.tile([128, T, 256], dt.bfloat16, tag="kvt")
                    nc.gpsimd.indirect_dma_start(
                        out=kv_t[:], out_offset=None, in_=kvq_table[:, :],
                        in_offset=IOA(ap=srcidx_sb[:, ic0:ic0 + T], axis=0),
                        element_offset=0)
                    q_t = pb.tile([128, T, 128], dt.bfloat16, tag="qt")
                    nc.gpsimd.indirect_dma_start(
                        out=q_t[:], out_offset=None, in_=kvq_table[:, :],
                        in_offset=IOA(ap=qdst_sb[:, ic0:ic0 + T], axis=0),
                        element_offset=256)
                    ke = kv_t[:, :, 0:128]
                    ve = kv_t[:, :, 128:256]
                    prod = pb.tile([128, T, 128], dt.bfloat16, tag="prod")
                    nc.vector.tensor_mul(out=prod[:], in0=ke, in1=q_t[:])
                    tval = pb.tile([128, T * 8], dt.float32, tag="tval")
                    nc.vector.tensor_reduce(
                        out=tval[:].rearrange("p (t h) -> p t h", h=H),
                        in_=prod[:].rearrange("p t (h d) -> p t h d", d=DK),
                        axis=mybir.AxisListType.X, op=AT.add)
                    texp = pb.tile([128, T * 8], dt.bfloat16, tag="texp")
                    nc.scalar.activation(texp[:], tval[:], AF.Exp, 0.0, 0.25)
                    msg = pb.tile([128, T, 128], dt.bfloat16, tag="msg")
                    texp_b = _bcast_last(
                        texp[:].rearrange("p (t h) -> p t h", h=H), DK)
                    nc.vector.tensor_tensor(
                        out=msg[:].rearrange("p t (h d) -> p t h d", d=DK),
                        in0=ve.rearrange("p t (h d) -> p t h d", d=DK),
                        in1=texp_b, op=AT.mult)
                    ps_s = pbps.tile([128, 8], dt.float32, tag="ps_s")
                    ps_agg = pbps.tile([128, 128], dt.float32, tag="ps_agg")
                    for i in range(T):
                        msel = pbm.tile([128, 128], dt.bfloat16, tag="msel")
                        nc.vector.tensor_scalar(
                            msel[:], iota_sb[:],
                            dloc_sb[:, ic0 + i:ic0 + i + 1], None,
                            AT.is_equal)
                        nc.tensor.matmul(out=ps_s[:], lhsT=msel[:],
                                         rhs=texp[:, i * 8:(i + 1) * 8],
                                         start=(i == 0), stop=(i == T - 1))
                        nc.tensor.matmul(out=ps_agg[:], lhsT=msel[:],
                                         rhs=msg[:, i, :],
                                         start=(i == 0), stop=(i == T - 1))
                    s_sb = pb.tile([128, 8], dt.float32, tag="s_sb")
                    nc.vector.tensor_scalar_max(s_sb[:], ps_s[:], 1e-6)
                    rec = pb.tile([128, 8], dt.float32, tag="rec")
                    nc.vector.reciprocal(rec[:], s_sb[:])
                    aggb = pb.tile([128, 128], dt.bfloat16, tag="aggb")
                    rec_b = _bcast_last(rec[:].rearrange("p h -> p h"), DK)
                    nc.vector.tensor_tensor(
                        out=aggb[:].rearrange("p (h d) -> p h d", d=DK),
                        in0=ps_agg[:].rearrange("p (h d) -> p h d", d=DK),
                        in1=rec_b, op=AT.mult)
                    ps_aT = pbps.tile([128, 128], dt.bfloat16, tag="ps_aT")
                    nc.tensor.transpose(out=ps_aT[:], in_=aggb[:],
                                        identity=identb_sb[:])
                    aggT = pb.tile([128, 128], dt.bfloat16, tag="aggT")
                    nc.vector.tensor_copy(out=aggT[:], in_=ps_aT[:])
                    ps_h = pbps.tile([128, 128], dt.float32, tag="ps_h")
                    nc.tensor.matmul(out=ps_h[:], lhsT=aggT[:], rhs=wa_sb[:],
                                     start=True, stop=False)
                    nc.tensor.matmul(out=ps_h[:], lhsT=ones_sb[:],
                                     rhs=ba_sb[:], start=False, stop=True)
                    hf = pb.tile([128, 128], dt.float32, tag="hf")
                    nc.scalar.activation(hf[:], ps_h[:], AF.Copy)
                    hb = pb.tile([128, 128], dt.bfloat16, tag="hb")
                    nc.vector.tensor_copy(out=hb[:], in_=ps_h[:])
                    nc.sync.dma_start(out=hout_f32[b * P:(b + 1) * P, :],
                                      in_=hf[:])
                    nc.sync.dma_start(out=hout_sh[b * P:(b + 1) * P, :],
                                      in_=hb[:])

            tc.strict_bb_all_engine_barrier()

            # ---------------- phase C: AllGather h_out (bf16)
            nc.gpsimd.collective_compute(
                "AllGather", AT.bypass,
                replica_groups=[list(range(NCORES))],
                ins=[hout_sh[:, :]], outs=[hout_full[:, :]])

            tc.strict_bb_all_engine_barrier()

            # ---------------- phase D: edge scores
            with tc.tile_pool(name="pd", bufs=2) as pd, \
                    tc.tile_pool(name="pdm", bufs=3) as pdm, \
                    tc.tile_pool(name="pdps", bufs=2, space="PSUM") as pdps:
                span = 0
                for (isrc, idst, odram) in ((spsrc_sb, spdst_sb, scores_pos),
                                            (snsrc_sb, sndst_sb, scores_neg)):
                    for c0 in range(0, SCOLS, 128):
                        ncols = min(128, SCOLS - c0)
                        cols_t = pd.tile([128, 128], dt.float32, tag="cols")
                        for g0 in range(0, ncols, 64):
                            gc = min(64, ncols - g0)
                            hs = pd.tile([128, 64, 128], dt.bfloat16,
                                         tag="hs")
                            nc.gpsimd.indirect_dma_start(
                                out=hs[:, :gc, :], out_offset=None,
                                in_=hout_full[:, :],
                                in_offset=IOA(
                                    ap=isrc[:, c0 + g0:c0 + g0 + gc],
                                    axis=0))
                            hd = pd.tile([128, 64, 128], dt.bfloat16,
                                         tag="hd")
                            nc.gpsimd.indirect_dma_start(
                                out=hd[:, :gc, :], out_offset=None,
                                in_=hout_full[:, :],
                                in_offset=IOA(
                                    ap=idst[:, c0 + g0:c0 + g0 + gc],
                                    axis=0))
                            for s0 in range(0, gc, 16):
                                sc = min(16, gc - s0)
                                prodd = pdm.tile([128, 16, 128], dt.bfloat16,
                                                 tag="pdprod")
                                nc.vector.tensor_mul(
                                    out=prodd[:, :sc, :],
                                    in0=hs[:, s0:s0 + sc, :],
                                    in1=hd[:, s0:s0 + sc, :])
                                cc = g0 + s0
                                if span % 2 == 0:
                                    nc.vector.tensor_reduce(
                                        out=cols_t[:, cc:cc + sc],
                                        in_=prodd[:, :sc, :],
                                        axis=mybir.AxisListType.X, op=AT.add)
                                else:
                                    junk = pdm.tile([128, 128], dt.bfloat16,
                                                    tag="junk")
                                    for jj in range(sc):
                                        nc.scalar.activation(
                                            junk[:], prodd[:, jj, :],
                                            AF.Copy,
                                            accum_out=cols_t[
                                                :, cc + jj:cc + jj + 1])
                                span += 1
                        ps_t = pdps.tile([128, 128], dt.float32, tag="ps_t")
                        nc.tensor.transpose(out=ps_t[:ncols, :],
                                            in_=cols_t[:, :ncols],
                                            identity=identf_sb[:])
                        osb = pd.tile([128, 128], dt.float32, tag="osb")
                        nc.vector.tensor_copy(out=osb[:ncols, :],
                                              in_=ps_t[:ncols, :])
                        nc.sync.dma_start(out=odram[c0:c0 + ncols, :],
                                          in_=osb[:ncols, :])

    nc.compile()
    return nc


# ---------------------------------------------------------------- entry

def _run(inputs, trace=False, sim=False, time_repeats=0):
    consts, per_core, meta = _prep(**inputs)
    T = meta["T"]
    key = ("v1", N, E, NBLK_C, T)
    if key not in _PROG_CACHE:
        _PROG_CACHE[key] = _build_program(T)
    nc = _PROG_CACHE[key]

    in_maps = []
    for c in range(NCORES):
        m = dict(consts)
        m.update(per_core[c])
        in_maps.append(m)

    if sim:
        from concourse.bass_interp import MultiCoreSim
        ms = MultiCoreSim(nc, num_cores=NCORES, trace=False,
                          require_finite=False, require_nnan=False)
        for c in range(NCORES):
            for k, v in in_maps[c].items():
                ms.cores[c].tensor(k)[:] = v
        ms.simulate()
        results = [{k: np.array(ms.cores[c].tensor(k))
                    for k in ("hout_f32", "scores_pos", "scores_neg")}
                   for c in range(NCORES)]
        br = None
    else:
        from concourse.bass_utils import run_bass_kernel_spmd
        br = run_bass_kernel_spmd(nc, in_maps, list(range(NCORES)),
                                  trace=trace)
        results = br.results
        if time_repeats:
            import time as _time
            walls = []
            for _ in range(time_repeats):
                t0 = _time.perf_counter()
                run_bass_kernel_spmd(nc, in_maps, list(range(NCORES)),
                                     trace=False)
                walls.append(int((_time.perf_counter() - t0) * 1e9))
            br.__dict__["wall_ns"] = walls

    node_of_row = meta["node_of_row"]
    hout_perm = np.concatenate([r["hout_f32"] for r in results], axis=0)
    h_out = np.zeros((N, OUT), np.float32)
    valid = node_of_row >= 0
    h_out[node_of_row[valid]] = hout_perm[valid]

    def scores(name):
        return np.concatenate(
            [np.asarray(r[name], np.float32).reshape(-1)[:SE_REAL]
             for r in results])

    sg = scores("scores_pos")
    sn = scores("scores_neg")
    return (h_out[:, None, :], sg[:, None, None], sn[:, None, None]), br


def kernel(**inputs):
    out, _ = _run(inputs, trace=False)
    return out
